# revision 7
# baseline (speedup 1.0000x reference)
"""EpisodicMemory kernel for Trainium2, data-parallel over batch on 8 NeuronCores.

Per-core computation (one batch element b, S=4096, D=1024, M=64, H=4, DH=256):

Host-side algebraic fusion (exact linear algebra, fp64 numpy):
  k        = mk @ wk.T + bk                              (M, D)
  FUSED_K  = stack_h[(k_h @ wq_h) / sqrt(DH)]            (H*M, D)
  scores   = x @ FUSED_K.T + sbias        (replaces q-proj + qk matmul)
  BIG_W    = [mk | wg | FUSED_K]                         (M+1+H*M, D)
  fused2   = comb_w[:, D:] @ out_w                       (D, D)
  cw1      = comb_w[:, :D]                               (D, D)
  combb    = comb_b + comb_w[:, D:] @ out_b              (D,)

Device algebra: fold fused2 into the value path per head:
  VF[(h,m), :] = v[m, hDH:(h+1)DH] @ fused2[:, hDH:(h+1)DH].T   (H*M, D)
  y = x @ cw1.T + P @ VF + combb     where P = concat_h softmax_h(scores)

Device phases (per core):
  1. per s-chunk (128 rows): pbig = x_chunk @ BIG_W.T -> [sim | gate | scores]
     one Exp over all 321 cols (logits are tiny -> no max subtraction),
     sigmoid gate via exp(-z) on the same ACT table (avoids table reloads),
     segmented per-head sums on DVE, normalize on GPSIMD, P transposed to
     pT_all via DMA XBAR transpose. W accumulated as gated.T @ [x8 | 1]
     with x8 an fp8 copy of x (write path tolerates fp8).
  2. slot_gate = min(colsum, 1); mv = slot_gate * W; v = mv @ wv.T + bv;
     VF per head via PE transposes + small matmuls.
  3. transposed output: for each d-chunk, yT[d, :] accumulates
     cw1T-chunks.T @ xT-stream + VF-chunks.T @ pT-stream in PSUM,
     + combb, written bf16 (host transposes back).
"""

import numpy as np
import ml_dtypes

import concourse.bass as bass
import concourse.mybir as mybir
import concourse.tile as tile
from concourse import bacc
from concourse.bass_utils import run_bass_kernel_spmd
from concourse.masks import make_identity

F32 = mybir.dt.float32
BF16 = mybir.dt.bfloat16
FP8 = mybir.dt.float8e4
AX = mybir.AxisListType.X
AF = mybir.ActivationFunctionType

B, D, M, H = 8, 1024, 64, 4
DH = D // H
GW = M + 1 + H * M  # 321 columns of BIG_W output
N_CORES = 8


def build_program(S=4096, add_sbias=False):
    NCH = S // 128   # s-chunks
    NT = S // 512    # s-tiles
    DC = D // 128    # d-chunks

    nc = bacc.Bacc(None, target_bir_lowering=False, debug=False)

    x8_d = nc.dram_tensor("x8", [S, D], FP8, kind="ExternalInput")
    xT_d = nc.dram_tensor("xT", [D, S], BF16, kind="ExternalInput")
    bigwT_d = nc.dram_tensor("bigwT", [D, GW], BF16, kind="ExternalInput")
    wvT_d = nc.dram_tensor("wvT", [D, D], BF16, kind="ExternalInput")
    f2T_d = nc.dram_tensor("f2T", [D, D], BF16, kind="ExternalInput")
    cw1T_d = nc.dram_tensor("cw1T", [D, D], BF16, kind="ExternalInput")
    bv_d = nc.dram_tensor("bv", [D], F32, kind="ExternalInput")
    combb_d = nc.dram_tensor("combb", [D], F32, kind="ExternalInput")
    wgbn_d = nc.dram_tensor("wgbn", [1], F32, kind="ExternalInput")
    sbias_d = nc.dram_tensor("sbias", [H * M], F32, kind="ExternalInput")
    yT_d = nc.dram_tensor("yT", [D, S], BF16, kind="ExternalOutput")

    x8_ap = x8_d.ap()
    yT_ap = yT_d.ap()
    xT_r = xT_d.ap().rearrange("(dc p) s -> p dc s", p=128)
    bigwT_r = bigwT_d.ap().rearrange("(dc p) g -> p dc g", p=128)
    wvT_r = wvT_d.ap().rearrange("(dc p) g -> p dc g", p=128)
    f2T_r = f2T_d.ap().rearrange("(dc p) g -> p dc g", p=128)
    cw1T_r = cw1T_d.ap().rearrange("(dc p) g -> p dc g", p=128)
    combb_r = combb_d.ap().rearrange("(dc p) -> p dc", p=128)

    def bcast(ap, n):
        return bass.AP(tensor=ap.tensor, offset=ap.offset, ap=[[0, n]] + list(ap.ap))

    with tile.TileContext(nc) as tc:
        with tc.tile_pool(name="singles", bufs=1) as singles:
            # phase-1-critical loads first, on the sync queue
            bigwT_sb = singles.tile([128, DC, GW], BF16)
            nc.sync.dma_start(bigwT_sb, bigwT_r)
            xT_sb = singles.tile([128, DC, S], BF16)
            for t in range(2):
                nc.sync.dma_start(xT_sb[:, :, t * 512:(t + 1) * 512],
                                  xT_r[:, :, t * 512:(t + 1) * 512])
            # later-phase weights on the gpsimd queue
            cw1T_sb = singles.tile([128, DC, D], BF16)
            nc.gpsimd.dma_start(cw1T_sb, cw1T_r)
            wvT_sb = singles.tile([128, DC, D], BF16)
            nc.gpsimd.dma_start(wvT_sb, wvT_r)
            f2T_sb = singles.tile([128, DC, D], BF16)
            nc.gpsimd.dma_start(f2T_sb, f2T_r)
            combb_sb = singles.tile([128, DC], F32)
            nc.gpsimd.dma_start(combb_sb, combb_r)
            bvb_sb = singles.tile([64, D], F32)
            nc.gpsimd.dma_start(bvb_sb, bcast(bv_d.ap(), 64))
            wgbn_sb = singles.tile([128, 1], F32)
            nc.gpsimd.dma_start(wgbn_sb, bcast(wgbn_d.ap(), 128))
            sbias_sb = singles.tile([128, H * M], F32)
            nc.gpsimd.dma_start(sbias_sb, bcast(sbias_d.ap(), 128))
            ident = singles.tile([128, 128], BF16)
            make_identity(nc, ident)
            ones_sb = singles.tile([128, 1], BF16)
            nc.vector.memset(ones_sb, 1.0)
            pT_all = singles.tile([128, 2, S], BF16)

            # ---------------- phase 1: write-attention ----------------
            with (
                tc.tile_pool(name="ps1", bufs=1, space="PSUM") as ps1,
                tc.tile_pool(name="xin", bufs=4) as xin,
                tc.tile_pool(name="wk1", bufs=3) as wk1,
            ):
                ps_w = ps1.tile([64, 1536], F32, tag="w")

                def issue_pbig(c):
                    pbig = ps1.tile([128, GW], F32, tag="big", bufs=2)
                    for dc in range(DC):
                        nc.tensor.matmul(
                            pbig,
                            lhsT=xT_sb[:, dc, c * 128:(c + 1) * 128],
                            rhs=bigwT_sb[:, dc, :],
                            start=(dc == 0), stop=(dc == DC - 1),
                        )
                    return pbig

                def process(c, pbig):
                    if add_sbias:
                        nc.vector.tensor_add(
                            pbig[:, M + 1:GW], pbig[:, M + 1:GW], sbias_sb
                        )
                    e_all = wk1.tile([128, GW], F32, tag="ea")
                    nc.scalar.activation(e_all, pbig, AF.Exp)
                    eg = wk1.tile([128, 1], F32, tag="eg")
                    nc.scalar.activation(eg, pbig[:, M:M + 1], AF.Exp,
                                         scale=-1.0, bias=wgbn_sb)
                    esum = wk1.tile([128, 1], F32, tag="esum")
                    nc.vector.reduce_sum(esum, e_all[:, 0:M], axis=AX)
                    hs = wk1.tile([128, H], F32, tag="hs")
                    nc.vector.reduce_sum(
                        hs, e_all[:, M + 1:GW].rearrange("p (h m) -> p h m", h=H),
                        axis=AX)
                    t1 = wk1.tile([128, 1], F32, tag="t1")
                    nc.vector.tensor_scalar_add(t1, eg, 1.0)
                    t2 = wk1.tile([128, 1], F32, tag="t2")
                    nc.vector.tensor_mul(t2, t1, esum)
                    scl = wk1.tile([128, 1], F32, tag="scl")
                    nc.vector.reciprocal(scl, t2)
                    rh = wk1.tile([128, H], F32, tag="rh")
                    nc.vector.reciprocal(rh, hs)
                    gc = wk1.tile([128, M], BF16, tag="gc")
                    nc.gpsimd.tensor_scalar_mul(gc, e_all[:, 0:M], scl)
                    pn = wk1.tile([128, H, M], BF16, tag="pn")
                    nc.gpsimd.tensor_mul(
                        pn, e_all[:, M + 1:GW].rearrange("p (h m) -> p h m", h=H),
                        rh.broadcast_to((128, H, M)))
                    pn2 = pn.rearrange("p h m -> p (h m)")
                    nc.sync.dma_start_transpose(
                        pT_all[:, 0, c * 128:(c + 1) * 128], pn2[:, 0:128])
                    nc.sync.dma_start_transpose(
                        pT_all[:, 1, c * 128:(c + 1) * 128], pn2[:, 128:256])
                    return gc

                def pe_tail(c, gc, xc):
                    st, sp = (c == 0), (c == NCH - 1)
                    nc.tensor.matmul(ps_w[:, 0:512], lhsT=gc,
                                     rhs=xc[:, 0:512], start=st, stop=sp)
                    nc.tensor.matmul(ps_w[:, 512:1024], lhsT=gc,
                                     rhs=xc[:, 512:1024], start=st, stop=sp)
                    nc.tensor.matmul(ps_w[:, 1024:1025], lhsT=gc,
                                     rhs=ones_sb, start=st, stop=sp)

                hist = []
                for c in range(NCH):
                    if c % 4 == 0 and c // 4 + 2 < NT:
                        t = c // 4 + 2
                        nc.sync.dma_start(
                            xT_sb[:, :, t * 512:(t + 1) * 512],
                            xT_r[:, :, t * 512:(t + 1) * 512])
                    xc = xin.tile([128, D], FP8, tag="xc")
                    nc.sync.dma_start(xc, x8_ap[c * 128:(c + 1) * 128, :])
                    pbig = issue_pbig(c)
                    hist.append((c, pbig, xc))
                    if c >= 1:
                        cc, pb, _ = hist[c - 1]
                        gc = process(cc, pb)
                        hist[c - 1] = (cc, gc, hist[c - 1][2])
                    if c >= 2:
                        cc, gc, xcc = hist[c - 2]
                        pe_tail(cc, gc, xcc)
                        hist[c - 2] = None
                c31, pb31, xc31 = hist[NCH - 1]
                gc31 = process(c31, pb31)
                c30, gc30, xc30 = hist[NCH - 2]
                pe_tail(c30, gc30, xc30)
                pe_tail(c31, gc31, xc31)

                # --- slot gate ---
                ssum = singles.tile([64, 1], F32)
                nc.vector.tensor_copy(ssum, ps_w[:, 1024:1025])
                sg = singles.tile([64, 1], F32)
                nc.vector.tensor_scalar_min(sg, ssum, 1.0)
                mv_bf = singles.tile([64, D], BF16)
                nc.vector.tensor_scalar_mul(mv_bf, ps_w[:, 0:D], sg)

            # ---------------- phase boundary: v and VF ----------------
            mvT_sb = singles.tile([128, DC, 64], BF16)
            vT_sb = singles.tile([128, DC, 64], BF16)
            v_sb = singles.tile([64, D], BF16)
            vf_sb = singles.tile([128, 2, D], BF16)
            with tc.tile_pool(name="psB", bufs=1, space="PSUM") as psB:
                for dc in range(DC):
                    ptr2 = psB.tile([128, 64], BF16, tag="tr2", bufs=2)
                    nc.tensor.transpose(
                        ptr2, mv_bf[:, dc * 128:(dc + 1) * 128],
                        ident[0:64, 0:64])
                    nc.vector.tensor_copy(mvT_sb[:, dc, :], ptr2)
                pv = psB.tile([64, D], F32, tag="v")
                for g2 in range(2):
                    for dc in range(DC):
                        nc.tensor.matmul(
                            pv[:, g2 * 512:(g2 + 1) * 512],
                            lhsT=mvT_sb[:, dc, :],
                            rhs=wvT_sb[:, dc, g2 * 512:(g2 + 1) * 512],
                            start=(dc == 0), stop=(dc == DC - 1),
                        )
                nc.vector.tensor_add(v_sb, pv, bvb_sb)
                for dc in range(DC):
                    ptr3 = psB.tile([128, 64], BF16, tag="tr2", bufs=2)
                    nc.tensor.transpose(
                        ptr3, v_sb[:, dc * 128:(dc + 1) * 128],
                        ident[0:64, 0:64])
                    nc.vector.tensor_copy(vT_sb[:, dc, :], ptr3)
                for q in range(2):
                    pvf = psB.tile([128, D], F32, tag="vf", bufs=2)
                    for hh in range(2):
                        h = 2 * q + hh
                        for cc in range(2):
                            for g2 in range(2):
                                nc.tensor.matmul(
                                    pvf[hh * 64:(hh + 1) * 64,
                                        g2 * 512:(g2 + 1) * 512],
                                    lhsT=vT_sb[:, h * 2 + cc, :],
                                    rhs=f2T_sb[:, h * 2 + cc,
                                               g2 * 512:(g2 + 1) * 512],
                                    start=(cc == 0), stop=(cc == 1),
                                )
                    nc.vector.tensor_copy(vf_sb[:, q, :], pvf)

            # ---------------- phase 2: transposed output ----------------
            with (
                tc.tile_pool(name="ps2", bufs=2, space="PSUM") as ps2,
                tc.tile_pool(name="wk2", bufs=4) as wk2,
            ):
                for dc in range(DC):
                    for sh in range(2):
                        pts = [ps2.tile([128, 512], F32, tag=f"o{st}",
                                        name=f"pt{st}")
                               for st in range(4)]
                        for ci in range(DC + 2):
                            if ci < DC:
                                lhsT = cw1T_sb[:, ci, dc * 128:(dc + 1) * 128]
                            else:
                                lhsT = vf_sb[:, ci - DC,
                                             dc * 128:(dc + 1) * 128]
                            for st in range(4):
                                s0 = sh * 2048 + st * 512
                                if ci < DC:
                                    rhs = xT_sb[:, ci, s0:s0 + 512]
                                else:
                                    rhs = pT_all[:, ci - DC, s0:s0 + 512]
                                nc.tensor.matmul(
                                    pts[st], lhsT=lhsT, rhs=rhs,
                                    start=(ci == 0), stop=(ci == DC + 1),
                                )
                        ytile = wk2.tile([128, 4, 512], BF16, tag="yt")
                        for st in range(4):
                            nc.vector.tensor_scalar_add(
                                ytile[:, st, :], pts[st],
                                combb_sb[:, dc:dc + 1])
                        nc.sync.dma_start(
                            yT_ap[dc * 128:(dc + 1) * 128,
                                  sh * 2048:(sh + 1) * 2048],
                            ytile,
                        )

    nc.compile()
    return nc


def prep_inputs(inputs, S=4096):
    """Host-side fusion + per-core shard maps."""
    f64 = np.float64
    bf = ml_dtypes.bfloat16
    f8 = ml_dtypes.float8_e4m3
    x = np.asarray(inputs["x"], np.float32)
    mk = np.asarray(inputs["memory_keys"], np.float32)
    wg_w = np.asarray(inputs["wg_w"], np.float32)
    wg_b = np.asarray(inputs["wg_b"], np.float32)
    ipw = np.asarray(inputs["in_proj_w"], np.float32)
    ipb = np.asarray(inputs["in_proj_b"], np.float32)
    out_w = np.asarray(inputs["out_w"], np.float32)
    out_b = np.asarray(inputs["out_b"], np.float32)
    comb_w = np.asarray(inputs["comb_w"], np.float32)
    comb_b = np.asarray(inputs["comb_b"], np.float32)

    wq, wk, wv = ipw[:D], ipw[D:2 * D], ipw[2 * D:]
    bq, bk, bv = ipb[:D], ipb[D:2 * D], ipb[2 * D:]

    k_full = mk.astype(f64) @ wk.astype(f64).T + bk.astype(f64)      # (M, D)
    kh = k_full.reshape(M, H, DH)
    wqh = wq.astype(f64).reshape(H, DH, D)
    scl = 1.0 / np.sqrt(DH)
    FK = (np.einsum("mhd,hde->hme", kh, wqh) * scl).reshape(H * M, D)
    sbias = (np.einsum("hd,mhd->hm", bq.astype(f64).reshape(H, DH), kh)
             * scl).reshape(H * M)
    BIG_W = np.concatenate([mk.astype(f64), wg_w.astype(f64), FK], axis=0)

    fused2 = comb_w[:, D:].astype(f64) @ out_w.astype(f64)           # (D, D)
    combb = comb_b.astype(f64) + comb_w[:, D:].astype(f64) @ out_b.astype(f64)

    shared = {
        "bigwT": np.ascontiguousarray(BIG_W.T).astype(bf),
        "wvT": np.ascontiguousarray(wv.T).astype(bf),
        "f2T": np.ascontiguousarray(fused2.T).astype(bf),
        "cw1T": np.ascontiguousarray(comb_w[:, :D].T).astype(bf),
        "bv": bv.astype(np.float32),
        "combb": combb.astype(np.float32),
        "wgbn": (-wg_b).astype(np.float32),
        "sbias": sbias.astype(np.float32),
    }
    add_sbias = bool(np.any(shared["sbias"] != 0))

    in_maps = []
    for b in range(B):
        xb = x[b, :S]
        m = dict(shared)
        m["x8"] = xb.astype(f8)
        m["xT"] = np.ascontiguousarray(xb.T).astype(bf)
        in_maps.append(m)
    return in_maps, add_sbias


def kernel(_trace=False, _S=4096, **inputs):
    in_maps, add_sbias = prep_inputs(inputs, S=_S)
    nc = build_program(S=_S, add_sbias=add_sbias)
    kw = {}
    if _trace:
        kw = dict(trace=True, trace_cores=list(range(N_CORES)))
    res = run_bass_kernel_spmd(nc, in_maps, list(range(N_CORES)), **kw)
    y = np.stack(
        [np.asarray(res.results[i]["yT"]).astype(np.float32).T
         for i in range(N_CORES)],
        axis=0,
    )
    if _trace:
        return y, res
    return y


# revision 9
# speedup vs baseline: 1.5693x; 1.5693x over previous
"""EpisodicMemory kernel for Trainium2, data-parallel over batch on 8 NeuronCores.

Per-core computation (one batch element b, S=4096, D=1024, M=64, H=4, DH=256):

Host-side algebraic fusion (exact linear algebra, fp64 numpy):
  k        = mk @ wk.T + bk                              (M, D)
  FUSED_K  = stack_h[(k_h @ wq_h) / sqrt(DH)]            (H*M, D)
  scores   = x @ FUSED_K.T + sbias        (replaces q-proj + qk matmul)
  BIG_W    = [mk | wg | FUSED_K]                         (M+1+H*M, D)
  fused2   = comb_w[:, D:] @ out_w                       (D, D)
  cw1      = comb_w[:, :D]                               (D, D)
  combb    = comb_b + comb_w[:, D:] @ out_b              (D,)

Device algebra: fold fused2 into the value path per head:
  VF[(h,m), :] = v[m, hDH:(h+1)DH] @ fused2[:, hDH:(h+1)DH].T   (H*M, D)
  y = x @ cw1.T + P @ VF + combb     where P = concat_h softmax_h(scores)

Device phases (per core):
  1. per s-chunk (128 rows): pbig = x_chunk @ BIG_W.T -> [sim | gate | scores]
     Exp-only ACT (tiny logits -> no max subtraction; sigmoid via exp(-z)),
     segmented per-head sums on DVE, write-gate on GPSIMD, P transposed on PE
     with one batched PSUM->SBUF copy. W accumulated as gated.T @ [x8 | 1]
     (x8 = fp8 copy of x; the write path tolerates fp8). Three-deep software
     pipeline so the PE never waits on the ACT/DVE softmax chain.
  2. slot_gate = min(colsum, 1); mv = slot_gate * W; v = mv @ wv.T + bv;
     VF per head via batched PE transposes + small matmuls.
  3. transposed output: for each d-chunk, yT[d, :] accumulates
     cw1T-chunks.T @ xT-stream + VF-chunks.T @ pT-stream in PSUM,
     + combb, written bf16 (host transposes back).
Weights are host-pre-shuffled to partition-major layouts for contiguous DMA.
"""

import numpy as np
import ml_dtypes

import concourse.bass as bass
import concourse.mybir as mybir
import concourse.tile as tile
from concourse import bacc
from concourse.bass_utils import run_bass_kernel_spmd
from concourse.masks import make_identity

F32 = mybir.dt.float32
BF16 = mybir.dt.bfloat16
FP8 = mybir.dt.float8e4
AX = mybir.AxisListType.X
AF = mybir.ActivationFunctionType
ALU = mybir.AluOpType

B, D, M, H = 8, 1024, 64, 4
DH = D // H
GW = M + 1 + H * M  # 321 columns of BIG_W output
N_CORES = 8


def build_program(S=4096, add_sbias=False):
    NCH = S // 128   # s-chunks
    NT = S // 512    # s-tiles
    DC = D // 128    # d-chunks

    nc = bacc.Bacc(None, target_bir_lowering=False, debug=False)

    x8_d = nc.dram_tensor("x8", [S, D], FP8, kind="ExternalInput")
    xT_d = nc.dram_tensor("xT", [D, S], BF16, kind="ExternalInput")
    bigwT_d = nc.dram_tensor("bigwT", [128, DC * GW], BF16, kind="ExternalInput")
    wvT_d = nc.dram_tensor("wvT", [128, DC * D], BF16, kind="ExternalInput")
    f2T_d = nc.dram_tensor("f2T", [128, DC * D], BF16, kind="ExternalInput")
    cw1T_d = nc.dram_tensor("cw1T", [128, DC * D], BF16, kind="ExternalInput")
    bv_d = nc.dram_tensor("bv", [D], F32, kind="ExternalInput")
    combb_d = nc.dram_tensor("combb", [D], F32, kind="ExternalInput")
    wgbn_d = nc.dram_tensor("wgbn", [1], F32, kind="ExternalInput")
    sbias_d = nc.dram_tensor("sbias", [H * M], F32, kind="ExternalInput")
    yT_d = nc.dram_tensor("yT", [D, S], BF16, kind="ExternalOutput")

    x8_ap = x8_d.ap()
    yT_ap = yT_d.ap()
    xT_r = xT_d.ap().rearrange("(dc p) s -> p dc s", p=128)
    bigwT_r = bigwT_d.ap().rearrange("p (dc g) -> p dc g", dc=DC)
    wvT_r = wvT_d.ap().rearrange("p (dc g) -> p dc g", dc=DC)
    f2T_r = f2T_d.ap().rearrange("p (dc g) -> p dc g", dc=DC)
    cw1T_r = cw1T_d.ap().rearrange("p (dc g) -> p dc g", dc=DC)
    combb_r = combb_d.ap().rearrange("(dc p) -> p dc", p=128)

    def bcast(ap, n):
        return bass.AP(tensor=ap.tensor, offset=ap.offset, ap=[[0, n]] + list(ap.ap))

    with tile.TileContext(nc) as tc:
        with tc.tile_pool(name="singles", bufs=1) as singles:
            # phase-1-critical loads first, on the sync queue
            bigwT_sb = singles.tile([128, DC, GW], BF16)
            nc.sync.dma_start(bigwT_sb, bigwT_r)
            xT_sb = singles.tile([128, DC, S], BF16)
            for q4 in range(4):
                nc.sync.dma_start(xT_sb[:, :, q4 * 128:(q4 + 1) * 128],
                                  xT_r[:, :, q4 * 128:(q4 + 1) * 128])
            nc.sync.dma_start(xT_sb[:, :, 512:1024], xT_r[:, :, 512:1024])
            # later-phase weights on the gpsimd queue
            cw1T_sb = singles.tile([128, DC, D], BF16)
            nc.gpsimd.dma_start(cw1T_sb, cw1T_r)
            wvT_sb = singles.tile([128, DC, D], BF16)
            nc.gpsimd.dma_start(wvT_sb, wvT_r)
            f2T_sb = singles.tile([128, DC, D], BF16)
            nc.gpsimd.dma_start(f2T_sb, f2T_r)
            combb_sb = singles.tile([128, DC], F32)
            nc.gpsimd.dma_start(combb_sb, combb_r)
            bvb_sb = singles.tile([64, D], F32)
            nc.gpsimd.dma_start(bvb_sb, bcast(bv_d.ap(), 64))
            wgbn_sb = singles.tile([128, 1], F32)
            nc.gpsimd.dma_start(wgbn_sb, bcast(wgbn_d.ap(), 128))
            sbias_sb = singles.tile([128, H * M], F32)
            nc.gpsimd.dma_start(sbias_sb, bcast(sbias_d.ap(), 128))
            ident = singles.tile([128, 128], BF16)
            make_identity(nc, ident)
            ones_sb = singles.tile([128, 1], BF16)
            nc.vector.memset(ones_sb, 1.0)
            pT_all = singles.tile([128, 2, S], BF16)

            # ---------------- phase 1: write-attention ----------------
            with (
                tc.tile_pool(name="ps1", bufs=1, space="PSUM") as ps1,
                tc.tile_pool(name="xin", bufs=5) as xin,
                tc.tile_pool(name="wk1", bufs=4) as wk1,
            ):
                ps_w = ps1.tile([64, 1536], F32, tag="w")

                def issue_pbig(c):
                    pbig = ps1.tile([128, GW], F32, tag="big", bufs=3)
                    for dc in range(DC):
                        nc.tensor.matmul(
                            pbig,
                            lhsT=xT_sb[:, dc, c * 128:(c + 1) * 128],
                            rhs=bigwT_sb[:, dc, :],
                            start=(dc == 0), stop=(dc == DC - 1),
                        )
                    return pbig

                def process(c, pbig):
                    if add_sbias:
                        nc.vector.tensor_add(
                            pbig[:, M + 1:GW], pbig[:, M + 1:GW], sbias_sb
                        )
                    e_sim = wk1.tile([128, M], F32, tag="es")
                    esum = wk1.tile([128, 1], F32, tag="esum")
                    nc.scalar.activation(e_sim, pbig[:, 0:M], AF.Exp,
                                         accum_out=esum)
                    eg = wk1.tile([128, 1], F32, tag="eg")
                    nc.scalar.activation(eg, pbig[:, M:M + 1], AF.Exp,
                                         scale=-1.0, bias=wgbn_sb)
                    eh = wk1.tile([128, H, M], F32, tag="eh")
                    nc.scalar.activation(eh, pbig[:, M + 1:GW], AF.Exp)
                    hs = wk1.tile([128, H], F32, tag="hs")
                    nc.vector.reduce_sum(hs, eh, axis=AX)
                    den = wk1.tile([128, 1], F32, tag="den")
                    nc.vector.scalar_tensor_tensor(
                        out=den, in0=eg, scalar=1.0, in1=esum,
                        op0=ALU.add, op1=ALU.mult)
                    scl = wk1.tile([128, 1], F32, tag="scl")
                    nc.vector.reciprocal(scl, den)
                    rh = wk1.tile([128, H], F32, tag="rh")
                    nc.vector.reciprocal(rh, hs)
                    gc = wk1.tile([128, M], BF16, tag="gc")
                    nc.gpsimd.tensor_scalar_mul(gc, e_sim, scl)
                    pn = wk1.tile([128, H, M], BF16, tag="pn")
                    nc.vector.tensor_mul(pn, eh, rh.broadcast_to((128, H, M)))

                    pn2 = pn.rearrange("p h m -> p (h m)")
                    ptr = ps1.tile([128, 256], BF16, tag="tr", bufs=2)
                    for j2 in range(2):
                        nc.tensor.transpose(
                            ptr[:, j2 * 128:(j2 + 1) * 128],
                            pn2[:, j2 * 128:(j2 + 1) * 128], ident)
                    nc.vector.tensor_copy(
                        pT_all[:, :, c * 128:(c + 1) * 128],
                        ptr.rearrange("p (j q) -> p j q", j=2))
                    return gc

                def pe_tail(c, gc, xc):
                    st, sp = (c == 0), (c == NCH - 1)
                    nc.tensor.matmul(ps_w[:, 0:512], lhsT=gc,
                                     rhs=xc[:, 0:512], start=st, stop=sp)
                    nc.tensor.matmul(ps_w[:, 512:1024], lhsT=gc,
                                     rhs=xc[:, 512:1024], start=st, stop=sp)
                    nc.tensor.matmul(ps_w[:, 1024:1025], lhsT=gc,
                                     rhs=ones_sb, start=st, stop=sp)

                DEPTH = 3
                hist = {}
                for c in range(NCH):
                    if c % 4 == 0 and c // 4 + 2 < NT:
                        t = c // 4 + 2
                        nc.sync.dma_start(
                            xT_sb[:, :, t * 512:(t + 1) * 512],
                            xT_r[:, :, t * 512:(t + 1) * 512])
                    xc = xin.tile([128, D], FP8, tag="xc")
                    nc.sync.dma_start(xc, x8_ap[c * 128:(c + 1) * 128, :])
                    pbig = issue_pbig(c)
                    hist[c] = [pbig, xc, None]
                    if c >= 1:
                        hist[c - 1][2] = process(c - 1, hist[c - 1][0])
                    if c >= DEPTH:
                        cc = c - DEPTH
                        pe_tail(cc, hist[cc][2], hist[cc][1])
                        del hist[cc]
                hist[NCH - 1][2] = process(NCH - 1, hist[NCH - 1][0])
                for cc in range(NCH - DEPTH, NCH):
                    pe_tail(cc, hist[cc][2], hist[cc][1])

                # --- slot gate ---
                ssum = singles.tile([64, 1], F32)
                nc.vector.tensor_copy(ssum, ps_w[:, 1024:1025])
                sg = singles.tile([64, 1], F32)
                nc.vector.tensor_scalar_min(sg, ssum, 1.0)
                mv_bf = singles.tile([64, D], BF16)
                nc.vector.tensor_scalar_mul(mv_bf, ps_w[:, 0:D], sg)

            # ---------------- phase boundary: v and VF ----------------
            mvT_sb = singles.tile([128, DC, 64], BF16)
            vT_sb = singles.tile([128, DC, 64], BF16)
            v_sb = singles.tile([64, D], BF16)
            vf_sb = singles.tile([128, 2, D], BF16)
            with tc.tile_pool(name="psB", bufs=1, space="PSUM") as psB:
                trB0 = psB.tile([128, DC, 64], BF16, tag="trb", bufs=2)
                for dc in range(DC):
                    nc.tensor.transpose(
                        trB0[:, dc, :], mv_bf[:, dc * 128:(dc + 1) * 128],
                        ident[0:64, 0:64])
                nc.vector.tensor_copy(mvT_sb, trB0)
                pv = psB.tile([64, D], F32, tag="v")
                for g2 in range(2):
                    for dc in range(DC):
                        nc.tensor.matmul(
                            pv[:, g2 * 512:(g2 + 1) * 512],
                            lhsT=mvT_sb[:, dc, :],
                            rhs=wvT_sb[:, dc, g2 * 512:(g2 + 1) * 512],
                            start=(dc == 0), stop=(dc == DC - 1),
                        )
                nc.vector.tensor_add(v_sb, pv, bvb_sb)
                trB1 = psB.tile([128, DC, 64], BF16, tag="trb", bufs=2)
                for dc in range(DC):
                    nc.tensor.transpose(
                        trB1[:, dc, :], v_sb[:, dc * 128:(dc + 1) * 128],
                        ident[0:64, 0:64])
                nc.vector.tensor_copy(vT_sb, trB1)
                for q in range(2):
                    pvf = psB.tile([128, D], F32, tag="vf", bufs=2)
                    for hh in range(2):
                        h = 2 * q + hh
                        for cc in range(2):
                            for g2 in range(2):
                                nc.tensor.matmul(
                                    pvf[hh * 64:(hh + 1) * 64,
                                        g2 * 512:(g2 + 1) * 512],
                                    lhsT=vT_sb[:, h * 2 + cc, :],
                                    rhs=f2T_sb[:, h * 2 + cc,
                                               g2 * 512:(g2 + 1) * 512],
                                    start=(cc == 0), stop=(cc == 1),
                                )
                    nc.vector.tensor_copy(vf_sb[:, q, :], pvf)

            # ---------------- phase 2: transposed output ----------------
            with (
                tc.tile_pool(name="ps2", bufs=2, space="PSUM") as ps2,
                tc.tile_pool(name="wk2", bufs=4) as wk2,
            ):
                for dc in range(DC):
                    for sh in range(2):
                        po = [ps2.tile([128, 1024], F32, tag=f"o{i}",
                                       name=f"po{i}") for i in range(2)]
                        for ci in range(DC + 2):
                            if ci < DC:
                                lhsT = cw1T_sb[:, ci, dc * 128:(dc + 1) * 128]
                            else:
                                lhsT = vf_sb[:, ci - DC,
                                             dc * 128:(dc + 1) * 128]
                            for st in range(4):
                                s0 = sh * 2048 + st * 512
                                if ci < DC:
                                    rhs = xT_sb[:, ci, s0:s0 + 512]
                                else:
                                    rhs = pT_all[:, ci - DC, s0:s0 + 512]
                                nc.tensor.matmul(
                                    po[st // 2][:, (st % 2) * 512:
                                                (st % 2) * 512 + 512],
                                    lhsT=lhsT, rhs=rhs,
                                    start=(ci == 0), stop=(ci == DC + 1),
                                )
                        ytile = wk2.tile([128, 4, 512], BF16, tag="yt")
                        for i in range(2):
                            nc.vector.tensor_scalar_add(
                                ytile[:, 2 * i:2 * i + 2, :], po[i],
                                combb_sb[:, dc:dc + 1])
                        nc.sync.dma_start(
                            yT_ap[dc * 128:(dc + 1) * 128,
                                  sh * 2048:(sh + 1) * 2048],
                            ytile,
                        )

    nc.compile()
    return nc


def prep_inputs(inputs, S=4096):
    """Host-side fusion + per-core shard maps."""
    f64 = np.float64
    bf = ml_dtypes.bfloat16
    f8 = ml_dtypes.float8_e4m3
    x = np.asarray(inputs["x"], np.float32)
    mk = np.asarray(inputs["memory_keys"], np.float32)
    wg_w = np.asarray(inputs["wg_w"], np.float32)
    wg_b = np.asarray(inputs["wg_b"], np.float32)
    ipw = np.asarray(inputs["in_proj_w"], np.float32)
    ipb = np.asarray(inputs["in_proj_b"], np.float32)
    out_w = np.asarray(inputs["out_w"], np.float32)
    out_b = np.asarray(inputs["out_b"], np.float32)
    comb_w = np.asarray(inputs["comb_w"], np.float32)
    comb_b = np.asarray(inputs["comb_b"], np.float32)

    wq, wk, wv = ipw[:D], ipw[D:2 * D], ipw[2 * D:]
    bq, bk, bv = ipb[:D], ipb[D:2 * D], ipb[2 * D:]

    k_full = mk.astype(f64) @ wk.astype(f64).T + bk.astype(f64)      # (M, D)
    kh = k_full.reshape(M, H, DH)
    wqh = wq.astype(f64).reshape(H, DH, D)
    scl = 1.0 / np.sqrt(DH)
    FK = (np.einsum("mhd,hde->hme", kh, wqh) * scl).reshape(H * M, D)
    sbias = (np.einsum("hd,mhd->hm", bq.astype(f64).reshape(H, DH), kh)
             * scl).reshape(H * M)
    BIG_W = np.concatenate([mk.astype(f64), wg_w.astype(f64), FK], axis=0)

    fused2 = comb_w[:, D:].astype(f64) @ out_w.astype(f64)           # (D, D)
    combb = comb_b.astype(f64) + comb_w[:, D:].astype(f64) @ out_b.astype(f64)

    def preshuffle(wT):
        # (D, G) -> (128, nc_*G): [p, dc*G+g] = wT[dc*128+p, g]
        nc_ = wT.shape[0] // 128
        G = wT.shape[1]
        return np.ascontiguousarray(
            wT.reshape(nc_, 128, G).transpose(1, 0, 2).reshape(128, nc_ * G))

    shared = {
        "bigwT": preshuffle(BIG_W.T).astype(bf),
        "wvT": preshuffle(np.ascontiguousarray(wv.T)).astype(bf),
        "f2T": preshuffle(np.ascontiguousarray(fused2.T)).astype(bf),
        "cw1T": preshuffle(np.ascontiguousarray(comb_w[:, :D].T)).astype(bf),
        "bv": bv.astype(np.float32),
        "combb": combb.astype(np.float32),
        "wgbn": (-wg_b).astype(np.float32),
        "sbias": sbias.astype(np.float32),
    }
    add_sbias = bool(np.any(shared["sbias"] != 0))

    in_maps = []
    for b in range(B):
        xb = x[b, :S]
        m = dict(shared)
        m["x8"] = xb.astype(f8)
        m["xT"] = np.ascontiguousarray(xb.T).astype(bf)
        in_maps.append(m)
    return in_maps, add_sbias


def kernel(_trace=False, _S=4096, **inputs):
    in_maps, add_sbias = prep_inputs(inputs, S=_S)
    nc = build_program(S=_S, add_sbias=add_sbias)
    kw = {}
    if _trace:
        kw = dict(trace=True, trace_cores=list(range(N_CORES)))
    res = run_bass_kernel_spmd(nc, in_maps, list(range(N_CORES)), **kw)
    y = np.stack(
        [np.asarray(res.results[i]["yT"]).astype(np.float32).T
         for i in range(N_CORES)],
        axis=0,
    )
    if _trace:
        return y, res
    return y
